# revision 4
# baseline (speedup 1.0000x reference)
"""LMUCell Trainium2 kernel (batch data-parallel over 8 NeuronCores).

Per core (batch shard of 4, T = 4*2048 = 8192 flattened time-batch steps):
  u   = tanh(x @ W_in^T + b_in)             [T, 1024]  (bf16 matmuls, fp32 acc)
  Bu  = u @ B^T                             [T, 512]
  scan c_t = A c_{t-1} + Bu_t  done as a 3-level chunked scan:
    phase A : 16 batched steps over 128-chunks/batch  (free dim 512)
    phase B1: 16 batched steps with A^16              (free dim 32)
    phase B2:  8 steps with A^256                     (free dim 4)
    (A^16, A^256 via repeated squaring in dual layout)
  h   = tanh(cT @ C^T + u_last @ D^T)       [4, 1024]

Layouts: feature-on-partitions ("transposed"); x is cast to bf16 in DRAM
(SWDGE cast DMA) then xbar-transposed into SBUF.
"""
import numpy as np
from contextlib import ExitStack

import concourse.bass as bass
import concourse.bacc as bacc
import concourse.tile as tile
import concourse.mybir as mybir
from concourse.bass_utils import run_bass_kernel_spmd

F32 = mybir.dt.float32
BF16 = mybir.dt.bfloat16
AF = mybir.ActivationFunctionType
ALU = mybir.AluOpType

NCORES = 8
BATCH, SEQ, INPUT, HIDDEN, MEM = 32, 2048, 1024, 1024, 512
BPC = BATCH // NCORES            # 4 batch elements per core
T = BPC * SEQ                    # 8192 flattened (b, s)
TC = 1024                        # phase-1 time chunk
NTC = T // TC                    # 8 chunks
K1, J1 = 16, 128                 # phase A: 16 steps, 128 chunks per batch elem
K2, J2 = 16, 8                   # phase B1
K3 = 8                           # phase B2
KI = INPUT // 128                # 8 input k-tiles
KH = HIDDEN // 128               # 8 hidden k-tiles
KM = MEM // 128                  # 4 mem tiles


def _mm_seq(nc, psum, pairs):
    """Accumulating matmul group into `psum` over (lhsT, rhs) pairs."""
    n = len(pairs)
    for i, (l, r) in enumerate(pairs):
        nc.tensor.matmul(psum, lhsT=l, rhs=r, start=(i == 0), stop=(i == n - 1))


def build():
    nc = bacc.Bacc(None, target_bir_lowering=False)
    x = nc.dram_tensor("x", [T, INPUT], F32, kind="ExternalInput")
    A = nc.dram_tensor("A", [MEM, MEM], F32, kind="ExternalInput")
    B = nc.dram_tensor("B", [MEM, HIDDEN], F32, kind="ExternalInput")
    C = nc.dram_tensor("C", [HIDDEN, MEM], F32, kind="ExternalInput")
    D = nc.dram_tensor("D", [HIDDEN, INPUT], F32, kind="ExternalInput")
    W = nc.dram_tensor("W_in", [HIDDEN, INPUT], F32, kind="ExternalInput")
    bvec = nc.dram_tensor("b_in", [128, KH], F32, kind="ExternalInput")
    h_out = nc.dram_tensor("h", [BPC, HIDDEN], F32, kind="ExternalOutput")

    xbf = nc.dram_tensor("xbf", [T, INPUT], BF16, kind="Internal")
    wbf = nc.dram_tensor("wbf", [HIDDEN, INPUT], BF16, kind="Internal")
    bbf = nc.dram_tensor("bbf", [MEM, HIDDEN], BF16, kind="Internal")
    abf = nc.dram_tensor("abf", [MEM, MEM], BF16, kind="Internal")
    cbf = nc.dram_tensor("cbf", [HIDDEN, MEM], BF16, kind="Internal")
    dbf = nc.dram_tensor("dbf", [HIDDEN, INPUT], BF16, kind="Internal")

    with tile.TileContext(nc) as tc, ExitStack() as ctx:
        const = ctx.enter_context(tc.tile_pool(name="const", bufs=1))

        # ---- casts to bf16 (SWDGE DMA, DRAM->DRAM) ----
        for t in range(NTC):
            nc.gpsimd.dma_start(xbf[t * TC:(t + 1) * TC, :], x[t * TC:(t + 1) * TC, :])
        nc.gpsimd.dma_start(wbf[:], W[:])
        nc.gpsimd.dma_start(bbf[:], B[:])
        nc.gpsimd.dma_start(abf[:], A[:])
        nc.gpsimd.dma_start(cbf[:], C[:])
        nc.gpsimd.dma_start(dbf[:], D[:])

        # ---- weights into SBUF (transposed layouts via xbar) ----
        # W^T: [i, h] as 8 ki-blocks of [128, 1024]
        WT = const.tile([128, KI * HIDDEN], BF16, name="WT")
        for k in range(KI):
            nc.sync.dma_start(WT[:, k * HIDDEN:(k + 1) * HIDDEN],
                              wbf[:, k * 128:(k + 1) * 128], transpose=True)
        # B^T: [h, m] as 8 kh-blocks of [128, 512]
        BT = const.tile([128, KH * MEM], BF16, name="BT")
        for k in range(KH):
            nc.sync.dma_start(BT[:, k * MEM:(k + 1) * MEM],
                              bbf[:, k * 128:(k + 1) * 128], transpose=True)
        # A^T and A natural: 4 blocks of [128, 512]
        AT = const.tile([128, KM * MEM], BF16, name="AT")
        AN = const.tile([128, KM * MEM], BF16, name="AN")
        for k in range(KM):
            nc.sync.dma_start(AT[:, k * MEM:(k + 1) * MEM],
                              abf[:, k * 128:(k + 1) * 128], transpose=True)
            nc.sync.dma_start(AN[:, k * MEM:(k + 1) * MEM],
                              abf[k * 128:(k + 1) * 128, :])
        # bias [128, 8]
        bsb = const.tile([128, KH], F32, name="bsb")
        nc.sync.dma_start(bsb[:], bvec[:])

        # persistent results of phase 1
        Bu = const.tile([128, KM * T], BF16, name="Bu")        # m-block major
        ulast = const.tile([128, KH * BPC], BF16, name="ulast")  # kh-blocks of [128, 4]

        # =================== phase 1: u and Bu ===================
        with ExitStack() as p1:
            xT_pool = p1.enter_context(tc.tile_pool(name="xT", bufs=2))
            u_pool = p1.enter_context(tc.tile_pool(name="u", bufs=2))
            psu = p1.enter_context(tc.tile_pool(name="psu", bufs=3, space="PSUM"))
            psb = p1.enter_context(tc.tile_pool(name="psb", bufs=3, space="PSUM"))

            for t in range(NTC):
                xT = xT_pool.tile([128, KI * TC], BF16, name="xTt", tag="xTt")
                for k in range(KI):
                    nc.sync.dma_start(xT[:, k * TC:(k + 1) * TC],
                                      xbf[t * TC:(t + 1) * TC, k * 128:(k + 1) * 128],
                                      transpose=True)
                ut = u_pool.tile([128, KH * TC], BF16, name="ut", tag="ut")
                for hj in range(KH):
                    for half in range(TC // 512):
                        ps = psu.tile([128, 512], F32, name="psu_t", tag="psu_t")
                        _mm_seq(nc, ps[:], [
                            (WT[:, k * HIDDEN + hj * 128: k * HIDDEN + hj * 128 + 128],
                             xT[:, k * TC + half * 512: k * TC + half * 512 + 512])
                            for k in range(KI)])
                        nc.scalar.activation(
                            ut[:, hj * TC + half * 512: hj * TC + half * 512 + 512],
                            ps[:], AF.Tanh, bias=bsb[:, hj:hj + 1], scale=1.0)
                if t % 2 == 1:
                    b = t // 2
                    for hj in range(KH):
                        nc.vector.tensor_copy(
                            ulast[:, hj * BPC + b: hj * BPC + b + 1],
                            ut[:, hj * TC + TC - 1: hj * TC + TC])
                for mj in range(KM):
                    for half in range(TC // 512):
                        ps = psb.tile([128, 512], F32, name="psb_t", tag="psb_t")
                        _mm_seq(nc, ps[:], [
                            (BT[:, k * MEM + mj * 128: k * MEM + mj * 128 + 128],
                             ut[:, k * TC + half * 512: k * TC + half * 512 + 512])
                            for k in range(KH)])
                        nc.vector.tensor_copy(
                            Bu[:, mj * T + t * TC + half * 512: mj * T + t * TC + half * 512 + 512],
                            ps[:])

        # =================== power chains (A^16, A^256) ===================
        # transposed-layout squaring:  Qn = Q^2 needs lhsT=N (natural), rhs=Q
        #                              Nn = N^2 needs lhsT=Q, rhs=N
        pows = ctx.enter_context(tc.tile_pool(name="pows", bufs=2))
        # single shared PSUM pool for chains + scan + readout (6 banks)
        pss = ctx.enter_context(tc.tile_pool(name="pss", bufs=6, space="PSUM"))

        def square(Q, N, qtag, ntag, want_nat, bufs=2):
            Qn = pows.tile([128, KM * MEM], BF16, name=qtag, tag=qtag, bufs=bufs)
            Nn = None
            for r in range(KM):
                ps = pss.tile([128, 512], F32, name="pst", tag="pst")
                _mm_seq(nc, ps[:], [
                    (N[:, k * MEM + r * 128: k * MEM + r * 128 + 128],
                     Q[:, k * MEM:(k + 1) * MEM]) for k in range(KM)])
                nc.vector.tensor_copy(Qn[:, r * MEM:(r + 1) * MEM], ps[:])
            if want_nat:
                Nn = pows.tile([128, KM * MEM], BF16, name=ntag, tag=ntag, bufs=bufs)
                for r in range(KM):
                    ps = pss.tile([128, 512], F32, name="pst", tag="pst")
                    _mm_seq(nc, ps[:], [
                        (Q[:, k * MEM + r * 128: k * MEM + r * 128 + 128],
                         N[:, k * MEM:(k + 1) * MEM]) for k in range(KM)])
                    nc.vector.tensor_copy(Nn[:, r * MEM:(r + 1) * MEM], ps[:])
            return Qn, Nn

        Q, N = AT, AN
        for s in range(3):                       # A^2..A^8 (ping-pong slots)
            Q, N = square(Q, N, "powq", "pown", True)
        BT2, BN2 = square(Q, N, "BT2", "BN2", True, bufs=1)   # A^16
        Q, N = BT2, BN2
        for s in range(3):                       # A^32..A^128
            Q, N = square(Q, N, "powq2", "pown2", True)
        CT2, _ = square(Q, N, "CT2", "", False, bufs=1)       # (A^256)^T

        # =================== scan ===================
        scan = ctx.enter_context(tc.tile_pool(name="scan", bufs=2))

        # phase A: 16 steps, state [m=4x128, (b=4, j=128)]
        SA = None
        for k in range(K1):
            SAn = scan.tile([128, KM * 512], BF16, name="SA", tag="SA")
            for r in range(KM):
                dst = SAn[:, r * 512:(r + 1) * 512].rearrange(
                    "p (b j) -> p b j", b=BPC)
                bu_sl = Bu[:, r * T:(r + 1) * T].rearrange(
                    "p (b j k) -> p b j k", b=BPC, k=K1)[:, :, :, k]
                if k == 0:
                    nc.vector.tensor_copy(dst, bu_sl)
                else:
                    ps = pss.tile([128, 512], F32, name="pst", tag="pst")
                    _mm_seq(nc, ps[:], [
                        (AT[:, kt * MEM + r * 128: kt * MEM + r * 128 + 128],
                         SA[:, kt * 512:(kt + 1) * 512]) for kt in range(KM)])
                    nc.vector.tensor_add(
                        dst, ps[:].rearrange("p (b j) -> p b j", b=BPC), bu_sl)
            SA = SAn

        # phase B1: 16 steps, state [m, (b=4, j2=8)] (free 32)
        SB = None
        for k in range(K2):
            SBn = scan.tile([128, KM * 32], BF16, name="SB", tag="SB")
            for r in range(KM):
                dst = SBn[:, r * 32:(r + 1) * 32].rearrange(
                    "p (b j) -> p b j", b=BPC)
                w_sl = SA[:, r * 512:(r + 1) * 512].rearrange(
                    "p (b j k) -> p b j k", b=BPC, k=K2)[:, :, :, k]
                if k == 0:
                    nc.vector.tensor_copy(dst, w_sl)
                else:
                    psw = pss.tile([128, 512], F32, name="pst", tag="pst")
                    ps = psw[:, 0:32]
                    _mm_seq(nc, ps, [
                        (BT2[:, kt * MEM + r * 128: kt * MEM + r * 128 + 128],
                         SB[:, kt * 32:(kt + 1) * 32]) for kt in range(KM)])
                    nc.vector.tensor_add(
                        dst, ps.rearrange("p (b j) -> p b j", b=BPC), w_sl)
            SB = SBn

        # phase B2: 8 steps, state [m, b=4]
        SC = None
        for k in range(K3):
            SCn = scan.tile([128, KM * BPC], BF16, name="SC", tag="SC")
            for r in range(KM):
                dst = SCn[:, r * BPC:(r + 1) * BPC]
                w_sl = SB[:, r * 32:(r + 1) * 32].rearrange(
                    "p (b j) -> p b j", b=BPC)[:, :, k]
                if k == 0:
                    nc.vector.tensor_copy(dst, w_sl)
                else:
                    psw = pss.tile([128, 512], F32, name="pst", tag="pst")
                    ps = psw[:, 0:BPC]
                    _mm_seq(nc, ps, [
                        (CT2[:, kt * MEM + r * 128: kt * MEM + r * 128 + 128],
                         SC[:, kt * BPC:(kt + 1) * BPC]) for kt in range(KM)])
                    nc.vector.tensor_add(dst, ps, w_sl)
            SC = SCn

        # =================== readout ===================
        # C^T: [m, hout] 4 blocks of [128, 1024];  D^T: [i, hout] 8 blocks
        CT = const.tile([128, KM * HIDDEN], BF16, name="CT")
        for k in range(KM):
            nc.sync.dma_start(CT[:, k * HIDDEN:(k + 1) * HIDDEN],
                              cbf[:, k * 128:(k + 1) * 128], transpose=True)
        DT = const.tile([128, KI * HIDDEN], BF16, name="DT")
        for k in range(KI):
            nc.sync.dma_start(DT[:, k * HIDDEN:(k + 1) * HIDDEN],
                              dbf[:, k * 128:(k + 1) * 128], transpose=True)

        out_pool = ctx.enter_context(tc.tile_pool(name="outp", bufs=1))
        h_view = h_out[:].rearrange("b (j p) -> j p b", p=128)
        for j in range(KH):
            psw = pss.tile([128, 512], F32, name="pst", tag="pst")
            ps = psw[:, 0:BPC]
            pairs = [(CT[:, kt * HIDDEN + j * 128: kt * HIDDEN + j * 128 + 128],
                      SC[:, kt * BPC:(kt + 1) * BPC]) for kt in range(KM)]
            pairs += [(DT[:, ki * HIDDEN + j * 128: ki * HIDDEN + j * 128 + 128],
                       ulast[:, ki * BPC:(ki + 1) * BPC]) for ki in range(KI)]
            _mm_seq(nc, ps, pairs)
            th = out_pool.tile([128, BPC], F32, name=f"th{j}")
            nc.scalar.activation(th[:], ps, AF.Tanh, scale=1.0)
            hf = out_pool.tile([128, BPC], F32, name=f"hf{j}")
            # NaN guard: hf = 0*psum + tanh(psum)  (keeps IEEE NaN/Inf exact)
            nc.vector.scalar_tensor_tensor(hf[:], ps, 0.0, th[:],
                                           op0=ALU.mult, op1=ALU.add)
            nc.sync.dma_start(h_view[j], hf[:])

    nc.finalize()
    return nc


_NC_CACHE = None


def _get_nc():
    global _NC_CACHE
    if _NC_CACHE is None:
        _NC_CACHE = build()
    return _NC_CACHE


def _in_maps(inputs):
    x = np.ascontiguousarray(np.asarray(inputs["x"], dtype=np.float32))
    A = np.ascontiguousarray(np.asarray(inputs["A"], dtype=np.float32))
    B = np.ascontiguousarray(np.asarray(inputs["B"], dtype=np.float32))
    C = np.ascontiguousarray(np.asarray(inputs["C"], dtype=np.float32))
    D = np.ascontiguousarray(np.asarray(inputs["D"], dtype=np.float32))
    W = np.ascontiguousarray(np.asarray(inputs["W_in"], dtype=np.float32))
    b = np.asarray(inputs["b_in"], dtype=np.float32)
    brearr = np.ascontiguousarray(b.reshape(KH, 128).T)   # [128, 8]
    maps = []
    for c in range(NCORES):
        xs = np.ascontiguousarray(
            x[c * BPC:(c + 1) * BPC].reshape(T, INPUT))
        maps.append({"x": xs, "A": A, "B": B, "C": C, "D": D,
                     "W_in": W, "b_in": brearr})
    return maps


def run(inputs, trace=False):
    nc = _get_nc()
    res = run_bass_kernel_spmd(nc, _in_maps(inputs),
                               core_ids=list(range(NCORES)), trace=trace)
    h = np.concatenate([res.results[c]["h"] for c in range(NCORES)], axis=0)
    return h.astype(np.float32), res


def kernel(**inputs) -> np.ndarray:
    h, _ = run(inputs, trace=False)
    return h
